# revision 28
# baseline (speedup 1.0000x reference)
"""Channel-wise min/max stats kernel for Trainium2 (8 NeuronCores).

Input:  tensor [1024, 32768] float32
Output: (min_vals [1024], max_vals [1024]) float32  -- per-channel min/max

Sharding: channel axis split across 8 cores (128 channels each -> exactly the
128 SBUF partitions). Each core reduces its own rows; host concatenates.
No collectives needed.

Per-core kernel (raw Bass, manual sems): the 16 MiB slice is streamed in
N_CHUNKS chunk DMAs into one resident SBUF buffer. Each chunk [128, CHUNK]
gets a tensor_reduce(min) and a tensor_reduce(max) into per-chunk partials
(DVE ingests 1 elem/cycle; min+max = 2 full passes = the DVE floor on this
toolchain -- fused 2-stream reduce ops don't compile and no other engine can
reduce along the free axis). Final tiny reduces collapse partials to [128,2],
one DMA out.
"""

import sys
from contextlib import ExitStack

for _p in ("/opt/trn_rl_repo",):
    if _p not in sys.path:
        sys.path.insert(0, _p)

import numpy as np

import concourse.bass as bass
import concourse.mybir as mybir
from concourse.bass_utils import run_bass_kernel_spmd

P = 128            # partitions = channels per core
W = 32768          # elements per channel
C = 1024           # total channels
N_CORES = 8
# Ramped chunk schedule: small first chunks so the DVE starts reducing as
# early as possible, large later chunks to amortize per-instruction overhead.
CHUNKS = [256, 256, 512, 512, 1024, 1024, 2048, 2048, 4096, 4096, 8192, 8704]
assert sum(CHUNKS) == W
N_CHUNKS = len(CHUNKS)
OFFS = [sum(CHUNKS[:j]) for j in range(N_CHUNKS)]


_NC_CACHE = {}


def _build_bass(sem_chain=False, detect_races=False):
    """Build the per-core program.

    sem_chain=True threads a semaphore through the DVE ops so CoreSim's race
    detector can verify the DMA<->DVE synchronization (the partials reuse
    between back-to-back DVE ops is safe on HW -- the DVE executes in order --
    but the detector can't know that). The production build omits the chain.
    """
    f32 = mybir.dt.float32
    nc = bass.Bass(detect_race_conditions=detect_races)
    x = nc.declare_dram_parameter("x", [P, W], f32, isOutput=False)
    mnmx_out = nc.declare_dram_parameter("mnmx", [P, 2], f32, isOutput=True)

    with ExitStack() as ctx:
        data = ctx.enter_context(nc.sbuf_tensor("data", [P, W], f32))
        mins = ctx.enter_context(nc.sbuf_tensor("mins", [P, N_CHUNKS], f32))
        maxs = ctx.enter_context(nc.sbuf_tensor("maxs", [P, N_CHUNKS], f32))
        mnmx = ctx.enter_context(nc.sbuf_tensor("mnmx_sb", [P, 2], f32))
        ld_sems = [
            ctx.enter_context(nc.semaphore(f"ld{j}")) for j in range(N_CHUNKS)
        ]
        sem_v = ctx.enter_context(nc.semaphore("vec_done"))
        sem_st = ctx.enter_context(nc.semaphore("st_done"))
        sem_ch = (
            ctx.enter_context(nc.semaphore("dve_chain")) if sem_chain else None
        )
        block = ctx.enter_context(nc.Block())

        # Loads issued from the ACT HWDGE ring: its engine preamble retires
        # slightly before the SP ring's, so the first chunk lands earlier.
        @block.scalar
        def _(scalar):
            for j in range(N_CHUNKS):
                sl = slice(OFFS[j], OFFS[j] + CHUNKS[j])
                scalar.dma_start(out=data[:, sl], in_=x[:, sl]).then_inc(
                    ld_sems[j], 16
                )

        @block.sync
        def _(sync):
            if sem_chain:
                sync.wait_ge(sem_ch, 2 * N_CHUNKS + 2)
            else:
                sync.wait_ge(sem_v, 1)
            sync.dma_start(out=mnmx_out[:], in_=mnmx[:]).then_inc(sem_st, 16)
            sync.wait_ge(sem_st, 16)

        @block.vector
        def _(vector):
            k = 0

            def chain_pre(vec):
                nonlocal k
                if sem_chain and k > 0:
                    vec.wait_ge(sem_ch, k)

            def chain_post(ins):
                nonlocal k
                if sem_chain:
                    ins.then_inc(sem_ch, 1)
                k += 1
                return ins

            for j in range(N_CHUNKS):
                sl = slice(OFFS[j], OFFS[j] + CHUNKS[j])
                vector.wait_ge(ld_sems[j], 16)
                for op, dst in (
                    (mybir.AluOpType.min, mins),
                    (mybir.AluOpType.max, maxs),
                ):
                    chain_pre(vector)
                    chain_post(nc.vector.tensor_reduce(
                        out=dst[:, j : j + 1],
                        in_=data[:, sl],
                        axis=mybir.AxisListType.X,
                        op=op,
                    ))
            chain_pre(vector)
            chain_post(nc.vector.tensor_reduce(
                out=mnmx[:, 0:1], in_=mins[:], axis=mybir.AxisListType.X,
                op=mybir.AluOpType.min,
            ))
            chain_pre(vector)
            ins = nc.vector.tensor_reduce(
                out=mnmx[:, 1:2], in_=maxs[:], axis=mybir.AxisListType.X,
                op=mybir.AluOpType.max,
            )
            chain_post(ins)
            if not sem_chain:
                ins.then_inc(sem_v, 1)

    return nc


def _get_nc():
    if "nc" not in _NC_CACHE:
        _NC_CACHE["nc"] = _build_bass()
    return _NC_CACHE["nc"]


def run(tensor, trace=False):
    """Run the SPMD kernel; returns (min_vals, max_vals, BassKernelResults)."""
    x = np.ascontiguousarray(np.asarray(tensor, dtype=np.float32))
    assert x.shape == (C, W), x.shape
    in_maps = [
        {"x": np.ascontiguousarray(x[i * P : (i + 1) * P])} for i in range(N_CORES)
    ]
    nc = _get_nc()
    out = run_bass_kernel_spmd(nc, in_maps, core_ids=list(range(N_CORES)), trace=trace)
    mins = np.concatenate([r["mnmx"][:, 0] for r in out.results])
    maxs = np.concatenate([r["mnmx"][:, 1] for r in out.results])
    return mins, maxs, out


def kernel(tensor):
    mins, maxs, _ = run(tensor, trace=False)
    return mins, maxs


# revision 29
# speedup vs baseline: 1.0008x; 1.0008x over previous
"""Channel-wise min/max stats kernel for Trainium2 (8 NeuronCores).

Input:  tensor [1024, 32768] float32
Output: (min_vals [1024], max_vals [1024]) float32  -- per-channel min/max

Sharding: channel axis split across 8 cores (128 channels each -> exactly the
128 SBUF partitions). Each core reduces its own rows; host concatenates.
No collectives needed.

Per-core kernel (raw Bass, manual sems): the 16 MiB slice is streamed in
N_CHUNKS chunk DMAs into one resident SBUF buffer. Each chunk [128, CHUNK]
gets a tensor_reduce(min) and a tensor_reduce(max) into per-chunk partials
(DVE ingests 1 elem/cycle; min+max = 2 full passes = the DVE floor on this
toolchain -- fused 2-stream reduce ops don't compile and no other engine can
reduce along the free axis). Final tiny reduces collapse partials to [128,2],
one DMA out.
"""

import sys
from contextlib import ExitStack

for _p in ("/opt/trn_rl_repo",):
    if _p not in sys.path:
        sys.path.insert(0, _p)

import numpy as np

import concourse.bass as bass
import concourse.mybir as mybir
from concourse.bass_utils import run_bass_kernel_spmd

P = 128            # partitions = channels per core
W = 32768          # elements per channel
C = 1024           # total channels
N_CORES = 8
# Ramped chunk schedule: small first chunks so the DVE starts reducing as
# early as possible, large later chunks to amortize per-instruction overhead.
CHUNKS = [256, 256, 512, 512, 1024, 1024, 2048, 2048, 4096, 4096, 8192, 8704]
assert sum(CHUNKS) == W
N_CHUNKS = len(CHUNKS)
OFFS = [sum(CHUNKS[:j]) for j in range(N_CHUNKS)]

_NC_CACHE = {}


def _build_bass(sem_chain=False, detect_races=False):
    """Build the per-core program.

    sem_chain=True threads a semaphore through the DVE ops so CoreSim's race
    detector can verify the DMA<->DVE synchronization (the partials reuse
    between back-to-back DVE ops is safe on HW -- the DVE executes in order --
    but the detector can't know that). The production build omits the chain.
    """
    f32 = mybir.dt.float32
    nc = bass.Bass(detect_race_conditions=detect_races)
    x = nc.declare_dram_parameter("x", [P, W], f32, isOutput=False)
    mnmx_out = nc.declare_dram_parameter("mnmx", [P, 2], f32, isOutput=True)

    with ExitStack() as ctx:
        data = ctx.enter_context(nc.sbuf_tensor("data", [P, W], f32))
        mins = ctx.enter_context(nc.sbuf_tensor("mins", [P, N_CHUNKS], f32))
        maxs = ctx.enter_context(nc.sbuf_tensor("maxs", [P, N_CHUNKS], f32))
        mnmx = ctx.enter_context(nc.sbuf_tensor("mnmx_sb", [P, 2], f32))
        ld_sems = [
            ctx.enter_context(nc.semaphore(f"ld{j}")) for j in range(N_CHUNKS)
        ]
        sem_v = ctx.enter_context(nc.semaphore("vec_done"))
        sem_st = ctx.enter_context(nc.semaphore("st_done"))
        sem_ch = (
            ctx.enter_context(nc.semaphore("dve_chain")) if sem_chain else None
        )
        block = ctx.enter_context(nc.Block())

        # Loads issued from the ACT HWDGE ring: its engine preamble retires
        # slightly before the SP ring's, so the first chunk lands earlier.
        @block.scalar
        def _(scalar):
            for j in range(N_CHUNKS):
                sl = slice(OFFS[j], OFFS[j] + CHUNKS[j])
                scalar.dma_start(out=data[:, sl], in_=x[:, sl]).then_inc(
                    ld_sems[j], 16
                )

        @block.sync
        def _(sync):
            if sem_chain:
                sync.wait_ge(sem_ch, 2 * N_CHUNKS + 2)
            else:
                sync.wait_ge(sem_v, 1)
            sync.dma_start(out=mnmx_out[:], in_=mnmx[:]).then_inc(sem_st, 16)
            sync.wait_ge(sem_st, 16)

        @block.vector
        def _(vector):
            k = 0

            def chain_pre(vec):
                nonlocal k
                if sem_chain and k > 0:
                    vec.wait_ge(sem_ch, k)

            def chain_post(ins):
                nonlocal k
                if sem_chain:
                    ins.then_inc(sem_ch, 1)
                k += 1
                return ins

            for j in range(N_CHUNKS):
                sl = slice(OFFS[j], OFFS[j] + CHUNKS[j])
                vector.wait_ge(ld_sems[j], 16)
                for op, dst in (
                    (mybir.AluOpType.min, mins),
                    (mybir.AluOpType.max, maxs),
                ):
                    chain_pre(vector)
                    chain_post(nc.vector.tensor_reduce(
                        out=dst[:, j : j + 1],
                        in_=data[:, sl],
                        axis=mybir.AxisListType.X,
                        op=op,
                    ))
            chain_pre(vector)
            chain_post(nc.vector.tensor_reduce(
                out=mnmx[:, 0:1], in_=mins[:], axis=mybir.AxisListType.X,
                op=mybir.AluOpType.min,
            ))
            chain_pre(vector)
            ins = nc.vector.tensor_reduce(
                out=mnmx[:, 1:2], in_=maxs[:], axis=mybir.AxisListType.X,
                op=mybir.AluOpType.max,
            )
            chain_post(ins)
            if not sem_chain:
                ins.then_inc(sem_v, 1)

    return nc


def _get_nc():
    if "nc" not in _NC_CACHE:
        _NC_CACHE["nc"] = _build_bass()
    return _NC_CACHE["nc"]


def run(tensor, trace=False):
    """Run the SPMD kernel; returns (min_vals, max_vals, BassKernelResults)."""
    x = np.ascontiguousarray(np.asarray(tensor, dtype=np.float32))
    assert x.shape == (C, W), x.shape
    in_maps = [
        {"x": np.ascontiguousarray(x[i * P : (i + 1) * P])} for i in range(N_CORES)
    ]
    nc = _get_nc()
    out = run_bass_kernel_spmd(nc, in_maps, core_ids=list(range(N_CORES)), trace=trace)
    mins = np.concatenate([r["mnmx"][:, 0] for r in out.results])
    maxs = np.concatenate([r["mnmx"][:, 1] for r in out.results])
    return mins, maxs, out


def kernel(tensor):
    mins, maxs, _ = run(tensor, trace=False)
    return mins, maxs



# revision 32
# speedup vs baseline: 1.0361x; 1.0352x over previous
"""Channel-wise min/max stats kernel for Trainium2 (8 NeuronCores).

Input:  tensor [1024, 32768] float32
Output: (min_vals [1024], max_vals [1024]) float32  -- per-channel min/max

Sharding: channel axis split across 8 cores (128 channels each -> exactly the
128 SBUF partitions). Each core reduces its own rows; host concatenates.
No collectives needed.

Per-core kernel (raw Bass, manual sems): the 16 MiB slice is streamed in
N_CHUNKS chunk DMAs into one resident SBUF buffer. Each chunk [128, CHUNK]
gets a tensor_reduce(min) and a tensor_reduce(max) into per-chunk partials
(DVE ingests 1 elem/cycle; min+max = 2 full passes = the DVE floor on this
toolchain -- fused 2-stream reduce ops don't compile and no other engine can
reduce along the free axis). Final tiny reduces collapse partials to [128,2],
one DMA out.
"""

import sys
from contextlib import ExitStack

for _p in ("/opt/trn_rl_repo",):
    if _p not in sys.path:
        sys.path.insert(0, _p)

import numpy as np

import concourse.bass as bass
import concourse.mybir as mybir
from concourse.bass_utils import run_bass_kernel_spmd

P = 128            # partitions = channels per core
W = 32768          # elements per channel
C = 1024           # total channels
N_CORES = 8
# Ramped chunk schedule: small first chunks so the DVE starts reducing as
# early as possible, large later chunks to amortize per-instruction overhead.
CHUNKS = [256, 256, 512, 512, 1024, 1024, 2048, 2048, 4096, 4096, 8192, 8704]
assert sum(CHUNKS) == W
N_CHUNKS = len(CHUNKS)
OFFS = [sum(CHUNKS[:j]) for j in range(N_CHUNKS)]

_NC_CACHE = {}


def _build_bass(sem_chain=False, detect_races=False):
    """Build the per-core program.

    sem_chain=True threads a semaphore through the DVE ops so CoreSim's race
    detector can verify the DMA<->DVE synchronization (the partials reuse
    between back-to-back DVE ops is safe on HW -- the DVE executes in order --
    but the detector can't know that). The production build omits the chain.
    """
    f32 = mybir.dt.float32
    nc = bass.Bass(detect_race_conditions=detect_races)
    x = nc.declare_dram_parameter("x", [P, W], f32, isOutput=False)
    mnmx_out = nc.declare_dram_parameter("mnmx", [P, 2], f32, isOutput=True)

    with ExitStack() as ctx:
        data = ctx.enter_context(nc.sbuf_tensor("data", [P, W], f32))
        mins = ctx.enter_context(nc.sbuf_tensor("mins", [P, N_CHUNKS], f32))
        maxs = ctx.enter_context(nc.sbuf_tensor("maxs", [P, N_CHUNKS], f32))
        mnmx = ctx.enter_context(nc.sbuf_tensor("mnmx_sb", [P, 2], f32))
        ld_sems = [
            ctx.enter_context(nc.semaphore(f"ld{j}")) for j in range(N_CHUNKS)
        ]
        sem_v = ctx.enter_context(nc.semaphore("vec_done"))
        sem_st = ctx.enter_context(nc.semaphore("st_done"))
        sem_ch = (
            ctx.enter_context(nc.semaphore("dve_chain")) if sem_chain else None
        )
        block = ctx.enter_context(nc.Block())

        # Loads issued from the ACT HWDGE ring: its engine preamble retires
        # slightly before the SP ring's, so the first chunk lands earlier.
        @block.scalar
        def _(scalar):
            for j in range(N_CHUNKS):
                sl = slice(OFFS[j], OFFS[j] + CHUNKS[j])
                scalar.dma_start(out=data[:, sl], in_=x[:, sl]).then_inc(
                    ld_sems[j], 16
                )

        @block.sync
        def _(sync):
            if sem_chain:
                sync.wait_ge(sem_ch, 2 * N_CHUNKS + 2)
            else:
                sync.wait_ge(sem_v, 1)
            sync.dma_start(out=mnmx_out[:], in_=mnmx[:]).then_inc(sem_st, 16)
            if sem_chain:
                # validation build keeps the explicit receipt wait
                sync.wait_ge(sem_st, 16)
            # production relies on the block-exit DGE drain to quiesce the
            # output DMA before NEFF completion

        @block.vector
        def _(vector):
            k = 0

            def chain_pre(vec):
                nonlocal k
                if sem_chain and k > 0:
                    vec.wait_ge(sem_ch, k)

            def chain_post(ins):
                nonlocal k
                if sem_chain:
                    ins.then_inc(sem_ch, 1)
                k += 1
                return ins

            for j in range(N_CHUNKS):
                sl = slice(OFFS[j], OFFS[j] + CHUNKS[j])
                vector.wait_ge(ld_sems[j], 16)
                for op, dst in (
                    (mybir.AluOpType.min, mins),
                    (mybir.AluOpType.max, maxs),
                ):
                    chain_pre(vector)
                    chain_post(nc.vector.tensor_reduce(
                        out=dst[:, j : j + 1],
                        in_=data[:, sl],
                        axis=mybir.AxisListType.X,
                        op=op,
                    ))
            chain_pre(vector)
            chain_post(nc.vector.tensor_reduce(
                out=mnmx[:, 0:1], in_=mins[:], axis=mybir.AxisListType.X,
                op=mybir.AluOpType.min,
            ))
            chain_pre(vector)
            ins = nc.vector.tensor_reduce(
                out=mnmx[:, 1:2], in_=maxs[:], axis=mybir.AxisListType.X,
                op=mybir.AluOpType.max,
            )
            chain_post(ins)
            if not sem_chain:
                ins.then_inc(sem_v, 1)

    return nc


def _get_nc():
    if "nc" not in _NC_CACHE:
        _NC_CACHE["nc"] = _build_bass()
    return _NC_CACHE["nc"]


def run(tensor, trace=False):
    """Run the SPMD kernel; returns (min_vals, max_vals, BassKernelResults)."""
    x = np.ascontiguousarray(np.asarray(tensor, dtype=np.float32))
    assert x.shape == (C, W), x.shape
    in_maps = [
        {"x": np.ascontiguousarray(x[i * P : (i + 1) * P])} for i in range(N_CORES)
    ]
    nc = _get_nc()
    out = run_bass_kernel_spmd(nc, in_maps, core_ids=list(range(N_CORES)), trace=trace)
    mins = np.concatenate([r["mnmx"][:, 0] for r in out.results])
    maxs = np.concatenate([r["mnmx"][:, 1] for r in out.results])
    return mins, maxs, out


def kernel(tensor):
    mins, maxs, _ = run(tensor, trace=False)
    return mins, maxs

